# revision 20
# baseline (speedup 1.0000x reference)
"""GAT (graph attention) full-graph kernel for 8 Trainium2 NeuronCores.

Strategy (dst-sharded graph parallel):
  Launch 1 (SPMD, node-sharded): core k projects its 12,500 nodes:
    feat = x @ W (bf16 matmul, fp32 out), el/er = x @ (W @ attn_{l,r}) (fp32).
  Host: assembles the replicated feat table [8*12544, 128]f32, routes every
    edge to the core owning its dst node, buckets edges by
    (dst-tile(128), src-quartile(25088 rows, int16-indexable)), pads buckets
    to 128-edge blocks with a common-across-cores structure, and gathers the
    per-edge el[src]/er[dst] values (launch-1 outputs; pure indexing).
  Launch 2 (SPMD, dst-sharded): per 128-edge block:
    dma_gather feat rows by src (512B rows),
    M_T[e,d] = (dstoff[e]==d) one-hot (DVE compare vs iota),
    ex = exp(leakyrelu(el+er)) (DVE+ACT),
    psum[d, 0:132] += M_T^T @ [feat*ex | ex] (PE, accumulated per dst tile),
    epilogue: out[d] = relu(mean_h(numer_h / s_h) + mean(bias)).

Self-contained: hardcodes problem shapes; host work is integer routing,
gathers of device-computed arrays, and concatenation (all float arithmetic
happens on-device).
"""

import numpy as np
import ml_dtypes

import concourse.bacc as bacc
import concourse.bass as bass
import concourse.mybir as mybir
import concourse.tile as tile
from concourse.bass_utils import run_bass_kernel_spmd
from concourse.bass_interp import get_hw_module
from concourse.library_config import mlp

F32 = mybir.dt.float32
BF16 = mybir.dt.bfloat16
I16 = mybir.dt.int16

# ---- problem constants ----
N = 100000
H = 4
C = 32
E = 1600000
IN = 256
HC = H * C            # 128
NEG = 0.2

NCORES = 8
NPC = N // NCORES     # 12500 nodes per core
TILES = 98            # ceil(12500/128)
NPAD = TILES * 128    # 12544
QROWS = 2 * NPAD      # 25088 table rows per src-quartile (int16-safe)
STILE = 4             # dst tiles per supertile (gather batching)
NSUP = (TILES + STILE - 1) // STILE  # 25

_cache = {}


# --------------------------------------------------------------------------
# Launch 1: sharded projection
# --------------------------------------------------------------------------
# Tile-chunking for batched DMA (4 chunks covering 98 tiles)
L1_CHUNKS = [25, 25, 24, 24]


def build_launch1(repeat=1):
    nc = bacc.Bacc("TRN2", target_bir_lowering=False, debug=False,
                   num_devices=NCORES)
    xt_d = nc.dram_tensor("xt", [IN, NPAD], F32, kind="ExternalInput")
    w_d = nc.dram_tensor("w", [IN, HC], F32, kind="ExternalInput")
    wt_d = nc.dram_tensor("wt", [HC, IN], F32, kind="ExternalInput")
    alr_d = nc.dram_tensor("alr", [HC, 2 * H], F32, kind="ExternalInput")
    feat_d = nc.dram_tensor("feat", [NPAD, HC], BF16, kind="ExternalOutput")
    elr_d = nc.dram_tensor("elr", [NPAD, 2 * H], BF16, kind="ExternalOutput")

    CMAX = max(L1_CHUNKS)
    with tile.TileContext(nc) as tc:
        with (
            tc.tile_pool(name="const", bufs=1) as cp,
            tc.tile_pool(name="xf", bufs=2) as xf,
            tc.tile_pool(name="xb", bufs=2) as xb,
            tc.tile_pool(name="ob", bufs=2) as ob,
            tc.tile_pool(name="ps", bufs=3, space="PSUM") as ps,
            tc.tile_pool(name="pslr", bufs=3, space="PSUM") as pslr,
            tc.tile_pool(name="psa", bufs=1, space="PSUM") as psa,
        ):
            # a_lr[i, :] = sum_j W[i, j] * A[j, :]  (fp32, 256x8)
            wt_sb = cp.tile([HC, IN], F32)
            nc.sync.dma_start(wt_sb[:], wt_d[:])
            alr_sb = cp.tile([HC, 2 * H], F32)
            nc.sync.dma_start(alr_sb[:], alr_d[:])
            a_sb = []
            for i in range(2):
                pa = psa.tile([128, 2 * H], F32, tag=f"pa{i}")
                nc.tensor.matmul(pa[:], wt_sb[:, i * 128:(i + 1) * 128],
                                 alr_sb[:], start=True, stop=True)
                asb = cp.tile([128, 2 * H], F32, tag=f"a{i}")
                nc.vector.tensor_copy(asb[:], pa[:])
                a_sb.append(asb)

            # W slabs (f32 -> bf16)
            wb = []
            for i in range(2):
                wf = cp.tile([128, HC], F32, tag=f"wf{i}")
                nc.sync.dma_start(wf[:], w_d[i * 128:(i + 1) * 128, :])
                wbi = cp.tile([128, HC], BF16, tag=f"wb{i}")
                nc.scalar.copy(wbi[:], wf[:])
                wb.append(wbi)

            for rep in range(repeat):
              t0 = 0
              for ci, ct in enumerate(L1_CHUNKS):
                w = ct * 128
                cs = slice(t0 * 128, t0 * 128 + w)
                x0 = xf.tile([128, CMAX * 128], F32, tag="x0")
                nc.sync.dma_start(x0[:, 0:w], xt_d[0:128, cs])
                x1 = xf.tile([128, CMAX * 128], F32, tag="x1")
                nc.sync.dma_start(x1[:, 0:w], xt_d[128:256, cs])
                # f32 -> bf16 converts alternate between ACT and DVE
                xb0 = xb.tile([128, CMAX * 128], BF16, tag="xb0")
                xb1 = xb.tile([128, CMAX * 128], BF16, tag="xb1")
                if ci % 2 == 0:
                    nc.scalar.copy(xb0[:, 0:w], x0[:, 0:w])
                    nc.vector.tensor_copy(xb1[:, 0:w], x1[:, 0:w])
                else:
                    nc.vector.tensor_copy(xb0[:, 0:w], x0[:, 0:w])
                    nc.scalar.copy(xb1[:, 0:w], x1[:, 0:w])

                fb = ob.tile([128, CMAX * HC], BF16, tag="fb")
                eb = ob.tile([128, CMAX * 2 * H], BF16, tag="eb")
                for j in range(ct):
                    o = j * 128
                    pf = ps.tile([128, HC], F32, tag="pf")
                    nc.tensor.matmul(pf[:], xb0[:, o:o + 128], wb[0][:],
                                     start=True, stop=False)
                    nc.tensor.matmul(pf[:], xb1[:, o:o + 128], wb[1][:],
                                     start=False, stop=True)
                    plr = pslr.tile([128, 2 * H], F32, tag="plr")
                    nc.tensor.matmul(plr[:], x0[:, o:o + 128], a_sb[0][:],
                                     start=True, stop=False)
                    nc.tensor.matmul(plr[:], x1[:, o:o + 128], a_sb[1][:],
                                     start=False, stop=True)
                    nc.scalar.copy(fb[:, j * HC:(j + 1) * HC], pf[:])
                    nc.vector.tensor_copy(
                        eb[:, j * 2 * H:(j + 1) * 2 * H], plr[:])

                nc.sync.dma_start(
                    feat_d[cs, :].rearrange("(t p) c -> p t c", p=128),
                    fb[:, 0:ct * HC].rearrange("p (t c) -> p t c", c=HC))
                nc.sync.dma_start(
                    elr_d[cs, :].rearrange("(t p) c -> p t c", p=128),
                    eb[:, 0:ct * 2 * H].rearrange(
                        "p (t c) -> p t c", c=2 * H))
                t0 += ct
    nc.compile()
    nc.m = get_hw_module(nc.m)
    return nc


# --------------------------------------------------------------------------
# Launch 2: edge phase.  meta = dict with bucket structure (common per core).
# --------------------------------------------------------------------------
def build_launch2(meta, repeat=1):
    nb = meta["nb"]                # [TILES][4] blocks per bucket
    NBS_MAX = meta["nbs_max"]      # max blocks in one supertile
    NBSQ_MAX = meta["nbsq_max"]    # max blocks in one (supertile, quartile)
    BTOT = meta["btot"]            # total blocks
    sup_base = meta["sup_base"]    # block col base per supertile
    TROWS = NCORES * NPAD

    nc = bacc.Bacc("TRN2", target_bir_lowering=False, debug=False,
                   num_devices=NCORES)
    table_d = nc.dram_tensor("table", [TROWS, HC], BF16, kind="ExternalInput")
    idxs_d = nc.dram_tensor("idxs", [128, BTOT * 8], I16, kind="ExternalInput")
    dstoff_d = nc.dram_tensor("dstoff", [128, BTOT], BF16, kind="ExternalInput")
    iota_d = nc.dram_tensor("iota", [128, NBSQ_MAX * 128], BF16,
                            kind="ExternalInput")
    elr_d = nc.dram_tensor("elr", [128, BTOT * 2 * H], BF16, kind="ExternalInput")
    bias_d = nc.dram_tensor("bias", [1, HC], F32, kind="ExternalInput")
    out_d = nc.dram_tensor("out", [NPAD, C], F32, kind="ExternalOutput")

    with tile.TileContext(nc) as tc:
        nc.gpsimd.load_library(mlp)
        with (
            tc.tile_pool(name="const", bufs=1) as cp,
            tc.tile_pool(name="sup", bufs=2) as sp,
            tc.tile_pool(name="bk", bufs=2) as bp,
            tc.tile_pool(name="ep", bufs=2) as ep,
            tc.tile_pool(name="pso", bufs=2 * STILE, space="PSUM") as pso,
        ):
            # iota_T[p, d*NBSQ_MAX + b] = d  (transposed iota: block-minor)
            iota = cp.tile([128, 128 * NBSQ_MAX], BF16)
            nc.sync.dma_start(iota[:], iota_d[:])

            # bias: mean over heads, broadcast to 128 partitions
            bsb = cp.tile([1, HC], F32)
            nc.sync.dma_start(bsb[:], bias_d[:])
            b01 = cp.tile([1, C], F32)
            nc.vector.tensor_add(b01[:], bsb[:, 0:C], bsb[:, C:2 * C])
            b23 = cp.tile([1, C], F32)
            nc.vector.tensor_add(b23[:], bsb[:, 2 * C:3 * C], bsb[:, 3 * C:4 * C])
            bsum = cp.tile([1, C], F32)
            nc.vector.tensor_add(bsum[:], b01[:], b23[:])
            bmean = cp.tile([1, C], F32)
            nc.vector.tensor_scalar_mul(bmean[:], bsum[:], 0.25)
            ones = cp.tile([1, 128], F32)
            nc.gpsimd.memset(ones[:], 1.0)
            pb = pso.tile([128, HC + H], F32, tag="pout")
            nc.tensor.matmul(pb[:, 0:C], ones[:], bmean[:], start=True, stop=True)
            biasb = cp.tile([128, C], F32)
            nc.vector.tensor_copy(biasb[:], pb[:, 0:C])

            for rep in range(repeat):
              for s in range(NSUP):
                ts = list(range(s * STILE, min((s + 1) * STILE, TILES)))
                nb_sq = [sum(nb[t][q] for t in ts) for q in range(4)]
                nbs = sum(nb_sq)
                if nbs == 0:
                    continue
                cb = sup_base[s]          # global block col base

                idx_sb = sp.tile([128, NBS_MAX * 8], I16, tag="idx")
                nc.sync.dma_start(idx_sb[:, 0:nbs * 8],
                                  idxs_d[:, cb * 8:(cb + nbs) * 8])
                doff_sb = sp.tile([128, NBS_MAX], BF16, tag="doff")
                nc.sync.dma_start(doff_sb[:, 0:nbs], dstoff_d[:, cb:cb + nbs])
                elr_sb = sp.tile([128, NBS_MAX, 2 * H], BF16, tag="elr")
                nc.sync.dma_start(
                    elr_sb[:, 0:nbs, :].rearrange("p b h -> p (b h)"),
                    elr_d[:, cb * 2 * H:(cb + nbs) * 2 * H])
                gbuf = sp.tile([128, NBS_MAX, HC], BF16, tag="gbuf")

                qb = [0, 0, 0, 0]
                acc = 0
                for q in range(4):
                    qb[q] = acc
                    acc += nb_sq[q]

                pouts = {}
                for t in ts:
                    pt_ = pso.tile([128, HC + H], F32, tag="pout", name=f"pout{t}")
                    pouts[t] = pt_
                done_b = {t: 0 for t in ts}
                total_b = {t: sum(nb[t]) for t in ts}

                for q in range(4):
                    nq = nb_sq[q]
                    if nq == 0:
                        continue
                    L = nq * 128
                    nc.gpsimd.dma_gather(
                        gbuf[:, qb[q]:qb[q] + nq, :],
                        table_d[q * QROWS:(q + 1) * QROWS, :],
                        idx_sb[:, qb[q] * 8:(qb[q] + nq) * 8],
                        L, L, HC, single_packet=False,
                    )
                    gq = gbuf[:, qb[q]:qb[q] + nq, :]
                    # ---- batched per (s, q) ----
                    # one-hot, transposed layout: mt[p, d, b] (packed last dim
                    # => DVE 2x mode).  matmul stationary = mt[:, :, b].
                    mt = bp.tile([128, 128 * NBSQ_MAX], BF16, tag="mt")
                    mt3 = mt[:, 0:128 * nq].rearrange(
                        "p (d b) -> p d b", b=nq)
                    doff_bc = bass.AP(
                        doff_sb.tensor, doff_sb[:, qb[q]:qb[q] + nq].offset,
                        [doff_sb[:].ap[0], [0, 128], [1, nq]])
                    iota3 = bass.AP(
                        iota.tensor, iota[:].offset,
                        [iota[:].ap[0], [NBSQ_MAX, 128], [1, nq]])
                    nc.vector.tensor_tensor(
                        out=mt3, in0=doff_bc, in1=iota3,
                        op=mybir.AluOpType.is_equal)
                    # e = el + er (DVE, bf16 2x), e2 = leakyrelu(e) (ACT Prelu:
                    # parametric_relu shares the act table with Exp -> no
                    # table reloads)
                    e_sb = bp.tile([128, NBSQ_MAX * H], BF16, tag="e")
                    nc.vector.tensor_tensor(
                        out=e_sb[:, 0:nq * H].rearrange(
                            "p (b h) -> p b h", h=H),
                        in0=elr_sb[:, qb[q]:qb[q] + nq, 0:H],
                        in1=elr_sb[:, qb[q]:qb[q] + nq, H:2 * H],
                        op=mybir.AluOpType.add)
                    e2 = bp.tile([128, NBSQ_MAX * H], F32, tag="e2")
                    nc.scalar.activation(
                        e2[:, 0:nq * H], e_sb[:, 0:nq * H],
                        mybir.ActivationFunctionType.Prelu, alpha=NEG)
                    # exd = exp(e2) broadcast-expanded to [128, nq, H*C] (ACT)
                    exd = bp.tile([128, NBSQ_MAX, HC], BF16, tag="exd")
                    e2_bc = bass.AP(
                        e2.tensor, e2[:].offset,
                        [e2[:].ap[0], [H, nq], [1, H], [0, C]])
                    exd4 = bass.AP(
                        exd.tensor, exd[:].offset,
                        [exd[:].ap[0], [HC, nq], [C, H], [1, C]])
                    nc.scalar.activation(exd4, e2_bc,
                                         mybir.ActivationFunctionType.Exp)
                    # rhs = [feat * exd | ex]  (all-bf16 packed -> 2x mode)
                    rhs = bp.tile([128, NBSQ_MAX, HC + H], BF16, tag="rhs")
                    nc.vector.tensor_tensor(
                        out=rhs[:, 0:nq, 0:HC], in0=gq, in1=exd[:, 0:nq, :],
                        op=mybir.AluOpType.mult)
                    # ex columns via a second (tiny) ACT Exp, not a DVE copy
                    nc.scalar.activation(
                        rhs[:, 0:nq, HC:HC + H],
                        e2[:, 0:nq * H].rearrange("p (b h) -> p b h", h=H),
                        mybir.ActivationFunctionType.Exp)
                    # aggregate into per-tile psums
                    for ti, t in enumerate(ts):
                        cnt = nb[t][q]
                        if cnt == 0:
                            continue
                        off = qb[q] + sum(nb[t2][q] for t2 in ts[:ti]) - qb[q]
                        for j in range(cnt):
                            jb = off + j
                            nc.tensor.matmul(
                                pouts[t][:], mt3[:, :, jb],
                                rhs[:, jb, :],
                                start=(done_b[t] == 0),
                                stop=(done_b[t] == total_b[t] - 1),
                                skip_group_check=True)
                            done_b[t] += 1

                # ---- epilogues (batched out-DMA per supertile) ----
                osup = ep.tile([128, STILE * C], F32, tag="osup")
                s4 = ep.tile([128, STILE * H], F32, tag="s4")
                for ti, t in enumerate(ts):
                    nc.vector.tensor_scalar(
                        out=s4[:, ti * H:(ti + 1) * H],
                        in0=pouts[t][:, HC:HC + H], scalar1=4.0,
                        scalar2=1e-20, op0=mybir.AluOpType.mult,
                        op1=mybir.AluOpType.add)
                srec = ep.tile([128, STILE * H], F32, tag="srec")
                nc.vector.reciprocal_approx_fast(
                    srec[:, 0:len(ts) * H], s4[:, 0:len(ts) * H])
                for ti, t in enumerate(ts):
                    pout = pouts[t]
                    scaled = ep.tile([128, H, C], F32, tag="scaled")
                    srec_bc = bass.AP(
                        srec.tensor, srec[:, ti * H:(ti + 1) * H].offset,
                        [srec[:].ap[0], [1, H], [0, C]])
                    nc.vector.tensor_tensor(
                        out=scaled[:],
                        in0=pout[:, 0:HC].rearrange("p (h c) -> p h c", c=C),
                        in1=srec_bc, op=mybir.AluOpType.mult)
                    hs = ep.tile([128, C], F32, tag="hs")
                    nc.vector.tensor_reduce(
                        hs[:], scaled[:].rearrange("p h c -> p c h"),
                        axis=mybir.AxisListType.X, op=mybir.AluOpType.add)
                    hb = ep.tile([128, C], F32, tag="hb")
                    nc.gpsimd.tensor_add(hb[:], hs[:], biasb[:])
                    nc.scalar.activation(
                        osup[:, ti * C:(ti + 1) * C], hb[:],
                        mybir.ActivationFunctionType.Relu)
                nc.sync.dma_start(
                    out_d[ts[0] * 128:(ts[-1] + 1) * 128, :].rearrange(
                        "(t p) c -> p t c", p=128),
                    osup[:, 0:len(ts) * C].rearrange("p (t c) -> p t c", c=C))
    nc.compile()
    nc.m = get_hw_module(nc.m)
    return nc


# --------------------------------------------------------------------------
# Host-side routing
# --------------------------------------------------------------------------
def balance_tiles(owner, dloc, q):
    """Assign each core's nodes to dst tiles so that per-(tile, src-quartile)
    edge counts stay <= 512 (4 blocks of 128) where possible.  Returns
    perm[NCORES, NPC]: original local node -> tile*128 + slot."""
    target = 4 * 128
    perm = np.zeros((NCORES, NPC), np.int64)
    for k in range(NCORES):
        m = owner == k
        dv = np.zeros((NPC, 4), np.int64)
        np.add.at(dv, (dloc[m], q[m]), 1)
        order = np.argsort(-dv.sum(1), kind="stable")
        L = np.zeros((TILES, 4), np.int64)
        cnt = np.zeros(TILES, np.int64)
        cap = np.full(TILES, 128, np.int64)
        cap[TILES - 1] = NPC - (TILES - 1) * 128
        assign = np.zeros(NPC, np.int64)
        for n in order:
            d = dv[n]
            pen = np.maximum(L + d - target, 0).sum(1).astype(np.float64)
            pen += (L + d).max(1) * 1e-6      # tie-break: keep tiles level
            pen[cnt >= cap] = np.inf
            t = int(np.argmin(pen))
            L[t] += d
            assign[n] = t
            cnt[t] += 1

        # swap refinement: move overflow out of >target buckets
        tile_nodes = [np.where(assign == t)[0] for t in range(TILES)]
        for _ in range(6):
            over = np.maximum(L - target, 0)
            if over.sum() == 0:
                break
            improved = False
            for t in np.argsort(-over.sum(1)):
                if over[t].sum() == 0:
                    continue
                nt = tile_nodes[t]
                dvt = dv[nt]                       # [nt, 4]
                pen_t0 = np.maximum(L[t] - target, 0).sum()
                # candidate partner tiles: least loaded in the worst quartile
                qw = int(np.argmax(over[t]))
                cand = np.argsort(L[:, qw])[:8]
                best = None
                for t2 in cand:
                    if t2 == t:
                        continue
                    nt2 = tile_nodes[t2]
                    dvt2 = dv[nt2]                 # [m, 4]
                    pen_20 = np.maximum(L[t2] - target, 0).sum()
                    # pairwise swap deltas: d = dvt2[m] - dvt[n]
                    dd = dvt2[None, :, :] - dvt[:, None, :]   # [n, m, 4]
                    p1 = np.maximum(L[t] + dd - target, 0).sum(2)
                    p2 = np.maximum(L[t2] - dd - target, 0).sum(2)
                    gain = (pen_t0 + pen_20) - (p1 + p2)
                    i, j = np.unravel_index(np.argmax(gain), gain.shape)
                    if gain[i, j] > 0 and (best is None or gain[i, j] > best[0]):
                        best = (gain[i, j], int(t2), int(i), int(j))
                if best is not None:
                    _, t2, i, j = best
                    n1 = tile_nodes[t][i]
                    n2 = tile_nodes[t2][j]
                    L[t] += dv[n2] - dv[n1]
                    L[t2] += dv[n1] - dv[n2]
                    tile_nodes[t][i] = n2
                    tile_nodes[t2][j] = n1
                    assign[n1], assign[n2] = t2, t
                    improved = True
            if not improved:
                break

        slots = np.zeros(TILES, np.int64)
        for t in range(TILES):
            nt = tile_nodes[t]
            perm[k, nt] = t * 128 + np.arange(len(nt))
    return perm


B1 = 4 * 128 - 8            # soft cap: bucket max stays a 4-block bucket
B2 = 5 * 128 - 8            # hard-ish cap: never exceed 5 blocks


def _pen(M):
    """Boundary-aware penalty on per-(tile, q) max-over-core counts."""
    return (np.maximum(M - B1, 0).sum(-1)
            + 1000.0 * np.maximum(M - B2, 0).sum(-1)
            + 1e-4 * M.sum(-1))


def balance_ranks(v):
    """Greedy assignment of ranks to tiles (<=128 ranks per tile) on the full
    per-core count state v[NPC, NCORES, 4], minimizing the number of
    128-blocks of the per-(tile, q) max-over-core counts.  Overflow beyond
    4 blocks is concentrated into few 5-block buckets."""
    order = np.argsort(-v.sum((1, 2)), kind="stable")
    L = np.zeros((TILES, NCORES, 4), np.float64)
    M = np.zeros((TILES, 4), np.float64)      # current max over cores
    cnt = np.zeros(TILES, np.int64)
    cap = np.full(TILES, 128, np.int64)
    cap[TILES - 1] = NPC - (TILES - 1) * 128
    assign = np.zeros(NPC, np.int64)
    pen0 = _pen(M)
    for n in order:
        Mn = np.maximum(M, (L + v[n]).max(1))          # [TILES, 4]
        pen = _pen(Mn) - pen0
        pen[cnt >= cap] = np.inf
        t = int(np.argmin(pen))
        L[t] += v[n]
        M[t] = L[t].max(0)
        pen0[t] = _pen(M[t])
        assign[n] = t
        cnt[t] += 1

    # swap refinement on the true objective
    tile_nodes = [list(np.where(assign == t)[0]) for t in range(TILES)]
    for _ in range(10):
        over = np.maximum(M - B1, 0)
        improved = False
        for t in np.argsort(-over.sum(1)):
            if over[t].sum() <= 0:
                continue
            nt = np.asarray(tile_nodes[t])
            qw = int(np.argmax(over[t]))
            cand = np.argsort(M[:, qw])[:6]
            best = None
            p_t0 = pen0[t]
            for t2 in cand:
                if t2 == t:
                    continue
                nt2 = np.asarray(tile_nodes[t2])
                p_20 = pen0[t2]
                # try all (i from t) x (j from t2) swaps
                d1 = L[t][None, None] - v[nt][:, None] + v[nt2][None, :]
                d2 = L[t2][None, None] + v[nt][:, None] - v[nt2][None, :]
                p1 = _pen(d1.max(2))
                p2 = _pen(d2.max(2))
                gain = (p_t0 + p_20) - (p1 + p2)
                i, j = np.unravel_index(np.argmax(gain), gain.shape)
                if gain[i, j] > 1e-9 and (best is None or gain[i, j] > best[0]):
                    best = (gain[i, j], int(t2), int(i), int(j))
            if best is not None:
                _, t2, i, j = best
                n1 = tile_nodes[t][i]
                n2 = tile_nodes[t2][j]
                L[t] += v[n2] - v[n1]
                L[t2] += v[n1] - v[n2]
                M[t] = L[t].max(0)
                M[t2] = L[t2].max(0)
                pen0[t] = _pen(M[t])
                pen0[t2] = _pen(M[t2])
                tile_nodes[t][i] = n2
                tile_nodes[t2][j] = n1
                assign[n1], assign[n2] = t2, t
                improved = True
        if not improved:
            break
    return assign


def route_edges(src, dst):
    """Bucket edges by (owner core, dst tile, src quartile); pad to common
    128-edge blocks.  Node -> core assignment is dealt by dst-degree and the
    rank -> tile map is shared across cores, so per-(tile, quartile) counts
    are nearly equal across cores (small max-over-cores padding).
    Returns meta + per-core index arrays."""
    src = src.astype(np.int64)
    dst = dst.astype(np.int64)

    # --- node -> (core, loc): deal by total dst-degree ---
    deg_tot = np.bincount(dst, minlength=N)
    order = np.argsort(-deg_tot, kind="stable")
    ncore = np.empty(N, np.int64)
    nloc = np.empty(N, np.int64)
    ncore[order] = np.arange(N) % NCORES
    nloc[order] = np.arange(N) // NCORES

    # table rows & src quartiles (quartile = src core pair, never split)
    rown = ncore * NPAD + nloc
    q = ncore[src] // 2
    idx16 = (rown[src] - q * QROWS).astype(np.int16)
    row = rown[src]
    drow = rown[dst]
    owner = ncore[dst]

    # --- per-node per-quartile in-degree, rank-sorted per core ---
    dv = np.zeros((N, 4), np.int64)
    np.add.at(dv, (dst, q), 1)
    # canonical in-core order: by (total, d0, d1, d2) desc
    key = (deg_tot * (200 ** 3) + dv[:, 0] * (200 ** 2)
           + dv[:, 1] * 200 + dv[:, 2]).astype(np.int64)
    rank = np.empty(N, np.int64)
    for k in range(NCORES):
        m = np.where(ncore == k)[0]
        m_sorted = m[np.argsort(-key[m], kind="stable")]
        rank[m_sorted] = np.arange(NPC)

    # per-rank per-core degree vectors, shared rank -> tile map
    vr = np.zeros((NPC, NCORES, 4), np.float64)
    vr[rank, ncore] = dv
    asg = balance_ranks(vr)

    # slots: nodes of a tile ordered by rank
    t_of_rank = asg
    slot_in_tile = np.empty(NPC, np.int64)
    for t in range(TILES):
        rs = np.where(t_of_rank == t)[0]
        slot_in_tile[rs] = np.arange(len(rs))
    node_slot = t_of_rank[rank] * 128 + slot_in_tile[rank]

    slot = node_slot[dst]
    t_id = slot >> 7
    doff = (slot & 127).astype(np.float32)

    # bucket key: (owner, supertile, quartile, tile)
    sidx = t_id // STILE
    key = ((owner * NSUP + sidx) * 4 + q) * TILES + t_id
    order = np.argsort(key, kind="stable")
    key_s = key[order]
    idx16_s = idx16[order]
    doff_s = doff[order]
    row_s = row[order]
    drow_s = drow[order]

    cnt = np.bincount((owner * TILES + t_id) * 4 + q,
                      minlength=NCORES * TILES * 4).reshape(NCORES, TILES, 4)
    nb = np.ceil(cnt.max(axis=0) / 128.0).astype(np.int64)   # [TILES, 4]
    nbmax = int(nb.max())
    btot = int(nb.sum())
    epad = btot * 128

    boff = np.zeros((TILES, 4), np.int64)
    sup_base = []
    nbs_max = 0
    nbsq_max = 0
    cur = 0
    for s in range(NSUP):
        sup_base.append(cur)
        ts = range(s * STILE, min((s + 1) * STILE, TILES))
        for qq in range(4):
            q0 = cur
            for t in ts:
                boff[t, qq] = cur
                cur += nb[t, qq]
            nbsq_max = max(nbsq_max, cur - q0)
        nbs_max = max(nbs_max, cur - sup_base[-1])
    assert cur == btot

    idx_all = np.zeros((NCORES, epad), np.int16)
    dst_all = np.full((NCORES, epad), -1.0, np.float32)
    srow_all = np.zeros((NCORES, epad), np.int64)   # padded table row of src
    drow_all = np.zeros((NCORES, epad), np.int64)   # padded table row of dst

    core_of = key_s // (NSUP * 4 * TILES)
    core_starts = np.searchsorted(core_of, np.arange(NCORES + 1))
    for k in range(NCORES):
        a, b = core_starts[k], core_starts[k + 1]
        kk = key_s[a:b]
        ub, inv, ucnt = np.unique(kk, return_inverse=True, return_counts=True)
        starts = np.zeros(len(ub), np.int64)
        starts[1:] = np.cumsum(ucnt)[:-1]
        rank = np.arange(b - a) - starts[inv]
        ut = ub % TILES
        uq = (ub // TILES) % 4
        base = boff[ut, uq] * 128
        pos = base[inv] + rank
        idx_all[k, pos] = idx16_s[a:b]
        dst_all[k, pos] = doff_s[a:b]
        srow_all[k, pos] = row_s[a:b]
        drow_all[k, pos] = drow_s[a:b]
        # pads: idx16 stays 0 (valid row of the quartile); srow/drow 0 (finite)

    # idx pads must be valid *within their quartile*: idx 0 maps to row
    # q*QROWS which exists for every quartile -> fine.

    # wrap idxs for dma_gather: within each (s, q) gather segment,
    # idx position i -> partition i%16 (replicated x8), col i//16
    idxs_host = np.zeros((NCORES, 128, btot * 8), np.int16)
    for s in range(NSUP):
        ts = range(s * STILE, min((s + 1) * STILE, TILES))
        for qq in range(4):
            tl = [t for t in ts if nb[t, qq] > 0]
            if not tl:
                continue
            a = boff[tl[0], qq] * 128
            L = int(sum(nb[t, qq] for t in tl)) * 128
            seg = idx_all[:, a:a + L]                       # [NCORES, L]
            wrap = seg.reshape(NCORES, L // 16, 16).transpose(0, 2, 1)
            cb = a // 16                                    # = block*8
            idxs_host[:, :, cb:cb + L // 16] = np.tile(wrap, (1, 8, 1))

    dst_host = dst_all.reshape(NCORES, btot, 128).transpose(0, 2, 1)
    dst_host = np.ascontiguousarray(dst_host.astype(ml_dtypes.bfloat16))

    meta = {
        "nb": nb.tolist(),
        "nbmax": nbmax,
        "nbs_max": int(nbs_max),
        "nbsq_max": int(nbsq_max),
        "btot": btot,
        "sup_base": sup_base,
    }
    return meta, idxs_host, dst_host, srow_all, drow_all, ncore, nloc, node_slot


# --------------------------------------------------------------------------
def kernel(x, src, dst, W, attn_l, attn_r, bias):
    x = np.asarray(x, dtype=np.float32)
    src = np.asarray(src)
    dst = np.asarray(dst)
    W = np.asarray(W, dtype=np.float32)
    attn_l = np.asarray(attn_l, dtype=np.float32)
    attn_r = np.asarray(attn_r, dtype=np.float32)
    bias = np.asarray(bias, dtype=np.float32)

    (meta, idxs_host, dst_host, srow_all, drow_all,
     ncore, nloc, node_slot) = route_edges(src, dst)

    # ---- launch 1 ----
    if "l1" not in _cache:
        _cache["l1"] = build_launch1()
    nc1 = _cache["l1"]

    xt = np.ascontiguousarray(x.T)                     # [256, 100000]
    alr = np.zeros((HC, 2 * H), np.float32)            # block-diag attn layout
    for h in range(H):
        alr[h * C:(h + 1) * C, h] = attn_l[h]
        alr[h * C:(h + 1) * C, H + h] = attn_r[h]
    wt = np.ascontiguousarray(W.T)                     # [128, 256]

    # node id for (core, loc): inverse of the dealt assignment
    node_of = np.empty((NCORES, NPC), np.int64)
    node_of[ncore, nloc] = np.arange(N)

    in1 = []
    for k in range(NCORES):
        xtk = np.zeros((IN, NPAD), np.float32)
        xtk[:, :NPC] = xt[:, node_of[k]]
        in1.append({"xt": xtk, "w": W, "wt": wt, "alr": alr})
    res1 = run_bass_kernel_spmd(nc1, in1, list(range(NCORES)))

    table = np.concatenate([res1.results[k]["feat"] for k in range(NCORES)])
    elr_g = np.concatenate([res1.results[k]["elr"] for k in range(NCORES)])
    el_g = elr_g[:, 0:H]
    er_g = elr_g[:, H:2 * H]

    # ---- launch 2 inputs ----
    key2 = (meta["btot"], meta["nbmax"], meta["nbs_max"], meta["nbsq_max"],
            tuple(tuple(r) for r in meta["nb"]))
    if ("l2", key2) not in _cache:
        _cache[("l2", key2)] = build_launch2(meta)
    nc2 = _cache[("l2", key2)]

    # transposed iota: iota[p, d*NBSQ_MAX + b] = d
    iota = np.repeat(np.arange(128, dtype=np.float32),
                     meta["nbsq_max"]).reshape(1, -1).repeat(128, 0)
    iota = np.ascontiguousarray(iota.astype(ml_dtypes.bfloat16))
    bias2 = bias.reshape(1, HC)
    btot = meta["btot"]

    in2 = []
    for k in range(NCORES):
        # per-edge el[src], er[dst] (block-partition layout [128, btot, 8])
        elr = np.empty((btot, 128, 2 * H), ml_dtypes.bfloat16)
        elr[:, :, 0:H] = el_g[srow_all[k]].reshape(btot, 128, H)
        elr[:, :, H:2 * H] = er_g[drow_all[k]].reshape(btot, 128, H)
        elr = np.ascontiguousarray(
            elr.transpose(1, 0, 2).reshape(128, btot * 2 * H))
        in2.append({
            "table": table,
            "idxs": idxs_host[k],
            "dstoff": dst_host[k],
            "iota": iota,
            "elr": elr,
            "bias": bias2,
        })
    res2 = run_bass_kernel_spmd(nc2, in2, list(range(NCORES)))

    out = np.empty((N, C), np.float32)
    for k in range(NCORES):
        m = ncore == k
        out[m] = res2.results[k]["out"][node_slot[m]]
    return out.astype(np.float32)

